# revision 1
# baseline (speedup 1.0000x reference)
"""Trainium2 Bass kernel for nn_Block_50130858279051 (dense transformer block).

Sharding: 8 cores = 2 batch groups x 4-way tensor parallel.
  - Within a group of 4 cores (one batch): each core computes LN1 (duplicated),
    QKV and attention for its 4 heads over all 2048 tokens; an 8-rank
    AllToAll ships each head's Y to the token owner (cross-batch recv blocks
    are neutralized by zero rows in the per-core proj weights); each core then
    does full proj + residual + LN2 + MLP + residual for its 512-token chunk.
  - Host concatenates the 8 chunks into the full [2, 2048, 1024] output.

All matmuls bf16 with fp32 PSUM accumulation; the residual path stays fp32.
LN gains/biases and the 1/sqrt(hd) attention scale are folded into weights on
the host (biases in this problem are all zero and are skipped on device).
LN normalization is fused into the PE transpose via a diag(rstd) rhs.
Softmax runs without max-subtraction (logits are O(5)); denominators come from
an appended ones-column on V; 1/l is broadcast across partitions with a K=1
outer-product matmul.
"""
import sys

sys.path.insert(0, "/opt/trn_rl_repo")

import numpy as np
import ml_dtypes
from contextlib import ExitStack

import concourse.bacc as bacc
import concourse.mybir as mybir
import concourse.tile as tile
from concourse.bass_utils import run_bass_kernel_spmd
from concourse.masks import make_identity

B, T, C, H, HD = 2, 2048, 1024, 16, 64
HID = 4 * C
P = 128
NCORES, TPG = 8, 4          # 2 groups x 4 cores
TCHUNK = T // TPG           # 512 tokens per core in the MLP phase
HPC = H // TPG              # 4 heads per core
CS = C // P                 # 8 channel subtiles
TT = T // P                 # 16 token tiles
NQ = T // 512               # 4 query slices of 512
NT2 = TCHUNK // P           # 4 token tiles in the chunk
NS = HID // P               # 32 hidden subtiles
f32, bf16, f16 = mybir.dt.float32, mybir.dt.bfloat16, mybir.dt.float16
BF = ml_dtypes.bfloat16
ACT = mybir.ActivationFunctionType

DEBUG = False


def build_nc(debug=False, sim_mode=False, do_p1=True, do_p2=True,
             skip_proj=False, skip_ln2=False, skip_fc=False,
             skip_fc2=False):
    nc = bacc.Bacc("TRN2", target_bir_lowering=False, debug=False,
                   num_devices=NCORES, num_swdge_queues=4)
    xb = nc.declare_dram_parameter("xb", [T, C], bf16, isOutput=False)
    xc = nc.declare_dram_parameter("xc", [TCHUNK, C], f32, isOutput=False)
    wq = nc.declare_dram_parameter("wq", [P, CS, 2 * P], bf16, isOutput=False)
    wk = nc.declare_dram_parameter("wk", [P, CS, 2 * P], bf16, isOutput=False)
    wv = nc.declare_dram_parameter("wv", [P, CS, 2 * P], bf16, isOutput=False)
    pw = nc.declare_dram_parameter("pw", [P, 2 * CS, C], bf16, isOutput=False)
    fw = nc.declare_dram_parameter("fw", [8, P, CS, 512], bf16, isOutput=False)
    f2w = nc.declare_dram_parameter("f2w", [16, P, 4, 512], bf16,
                                isOutput=False)
    out = nc.declare_dram_parameter("out", [TCHUNK, C], f32, isOutput=True)
    dbg = {}
    if debug:
        dbg["xlt"] = nc.declare_dram_parameter("dbg_xlt", [P, CS, T], bf16,
                                               isOutput=True)
        dbg["q0"] = nc.declare_dram_parameter("dbg_q0", [P, T], bf16,
                                              isOutput=True)
        dbg["k0"] = nc.declare_dram_parameter("dbg_k0", [P, T], bf16,
                                              isOutput=True)
        dbg["v"] = nc.declare_dram_parameter("dbg_v", [P, TT, HPC, 65], bf16,
                                             isOutput=True)
        dbg["y"] = nc.declare_dram_parameter("dbg_y", [64, HPC, T], bf16,
                                             isOutput=True)
        dbg["z"] = nc.declare_dram_parameter("dbg_z", [T, C], f16,
                                             isOutput=True)
        dbg["x2"] = nc.declare_dram_parameter("dbg_x2", [P, NT2, C], f32,
                                              isOutput=True)
        dbg["ht"] = nc.declare_dram_parameter("dbg_ht", [P, NS, TCHUNK], bf16,
                                              isOutput=True)

    with tile.TileContext(nc) as tc, ExitStack() as ctx:
        per = ctx.enter_context(tc.tile_pool(name="persist", bufs=1))
        wpool = ctx.enter_context(tc.tile_pool(name="wpool", bufs=1))
        work = ctx.enter_context(tc.tile_pool(name="work", bufs=3))
        small = ctx.enter_context(tc.tile_pool(name="small", bufs=3))
        psmm = ctx.enter_context(
            tc.tile_pool(name="psmm", bufs=3, space="PSUM"))
        dram = ctx.enter_context(tc.tile_pool(name="dram", bufs=1,
                                              space="DRAM"))

        # ---- constants -------------------------------------------------
        ident = per.tile([P, P], bf16, tag="ident")
        make_identity(nc, ident[:])
        eps_t = per.tile([P, 1], f32, tag="eps")
        nc.gpsimd.memset(eps_t[:], 1e-5)


        # weight stream pools opened early so the first chunks prefetch
        # during phase 1; chunks rotate across SWDGE/HWDGE queues.
        fws = ctx.enter_context(tc.tile_pool(name="fwstream", bufs=4))
        f2s = ctx.enter_context(tc.tile_pool(name="f2stream", bufs=3))
        engs = [nc.gpsimd, nc.gpsimd, nc.sync, nc.scalar]
        fw_chunks = []
        for g in range(8):
            t = fws.tile([P, CS, 512], bf16, tag="fwch", name=f"fwch{g}")
            fw_chunks.append(t)
            if g < 4:
                engs[g % 4].dma_start(t[:], fw[g])
        f2_chunks = []
        for idx in range(16):
            n, sg = idx // 8, idx % 8
            t = f2s.tile([P, 4, 512], bf16, tag="f2ch", name=f"f2ch{idx}")
            f2_chunks.append(t)
            if idx < 3:
                engs[idx % 4].dma_start(t[:], f2w[idx])

        # 8-rank AllToAll (4-rank mesh unsupported): send-block c carries my
        # heads' Y for token chunk (c %% 4); recv-block j = core j's heads for
        # my chunk. Cross-batch blocks are neutralized by zero rows in pw.
        ybounce = dram.tile([NCORES * 256, TCHUNK], bf16)
        a2a_out = dram.tile([NCORES * 256, TCHUNK], bf16)

        def layernorm_transpose(get_tile, n_tiles, dst):
            """Token-major fp32 [P, C] tiles -> feature-major bf16 dst.

            dst[:, c, i*P:(i+1)*P] = ((x - mu) * rstd)^T for token tile i.
            """
            for i in range(n_tiles):
                xt = get_tile(i)
                s_sum = small.tile([P, 1], f32, tag="s_sum")
                s_sq = small.tile([P, 1], f32, tag="s_sq")
                tmp = small.tile([P, 1], f32, tag="s_tmp")
                negmu = small.tile([P, 1], f32, tag="s_negmu")
                rstd = small.tile([P, 1], f32, tag="s_rstd")
                cen = work.tile([P, C], bf16, tag="cen")
                # sum(x^2) per token (x^2 lands in cen as scratch)
                nc.scalar.activation(cen[:], xt[:], ACT.Square,
                                     accum_out=s_sq[:])
                nc.vector.tensor_reduce(out=s_sum[:], in_=xt[:],
                                        op=mybir.AluOpType.add,
                                        axis=mybir.AxisListType.X)
                nc.vector.tensor_scalar_mul(negmu[:], s_sum[:], -1.0 / C)
                nc.vector.tensor_mul(tmp[:], negmu[:], negmu[:])
                # tmp = sq/C - mu^2  (biased variance)
                nc.vector.scalar_tensor_tensor(
                    out=tmp[:], in0=s_sq[:], scalar=1.0 / C, in1=tmp[:],
                    op0=mybir.AluOpType.mult, op1=mybir.AluOpType.subtract)
                nc.scalar.activation(tmp[:], tmp[:], ACT.Sqrt, bias=eps_t[:])
                nc.vector.reciprocal(rstd[:], tmp[:])
                # centered bf16 copy: cen = x - mu
                nc.scalar.activation(cen[:], xt[:], ACT.Identity,
                                     bias=negmu[:])
                dmat = work.tile([P, P], bf16, tag="dmat")
                nc.vector.tensor_scalar_mul(dmat[:], ident[:], rstd[:])
                for half in range(2):
                    ps = psmm.tile([P, 512], f32, tag="mm")
                    for cq in range(4):
                        c = half * 4 + cq
                        nc.tensor.matmul(
                            ps[:, cq * P:(cq + 1) * P],
                            lhsT=cen[:, c * P:(c + 1) * P],
                            rhs=dmat[:], start=True, stop=True)
                    nc.vector.tensor_copy(
                        out=dst[:, half * 4:(half + 1) * 4,
                                i * P:(i + 1) * P],
                        in_=ps[:].rearrange("p (c t) -> p c t", c=4))

        # ================= phase 1: LN1, QKV, attention, proj ===========
        with tc.tile_pool(name="xlt_pool", bufs=1) as xlt_pool, \
             tc.tile_pool(name="at_pool", bufs=4) as at_pool, \
             tc.tile_pool(name="psy", bufs=2, space="PSUM") as psy, \
             tc.tile_pool(name="psatt", bufs=3, space="PSUM") as psatt:
            xlt = xlt_pool.tile([P, CS, T], bf16, tag="xlt")
            # multiplicative causal masks for the 4 diagonal positions.
            # A'^T tile at k-tile kt, q-slice qs (j = kt - 4*qs):
            # keep (1.0) iff ki - qj + 128*j <= 0
            masks = xlt_pool.tile([P, 4, 512], bf16, tag="masks")
            for j in range(4):
                m = masks[:, j, :]
                nc.gpsimd.memset(m, 1.0)
                nc.gpsimd.affine_select(
                    out=m, in_=m, compare_op=mybir.AluOpType.is_ge,
                    fill=0.0, base=-128 * j, pattern=[[1, 512]],
                    channel_multiplier=-1)
            ones64 = xlt_pool.tile([1, 64], bf16, tag="ones64")
            nc.gpsimd.memset(ones64[:], 1.0)
            vA = xlt_pool.tile([P, TT, HPC, 65], bf16, tag="vA")
            nc.gpsimd.memset(vA[:, :, :, 64:65], 1.0)
            qT = [xlt_pool.tile([P, T], bf16, tag=f"qT{p}", name=f"qT{p}")
                  for p in range(2)]
            kT = [xlt_pool.tile([P, T], bf16, tag=f"kT{p}", name=f"kT{p}")
                  for p in range(2)]
            ysb = [xlt_pool.tile([64, T], bf16, tag=f"ysb{h}",
                                 name=f"ysb{h}") for h in range(HPC)]
            wq_s = xlt_pool.tile([P, CS, 2 * P], bf16, tag="wq")
            wk_s = xlt_pool.tile([P, CS, 2 * P], bf16, tag="wk")
            wv_s = xlt_pool.tile([P, CS, 2 * P], bf16, tag="wv")
            nc.gpsimd.dma_start(wq_s[:], wq[:])
            nc.gpsimd.dma_start(wk_s[:], wk[:])
            nc.gpsimd.dma_start(wv_s[:], wv[:])

            def xb_tile(i):
                xt = work.tile([P, C], bf16, tag="xbbt")
                eng = nc.sync if i % 2 == 0 else nc.scalar
                eng.dma_start(xt[:], xb[i * P:(i + 1) * P, :])
                return xt

            if do_p1:
                layernorm_transpose(xb_tile, TT, xlt)

            # qkv + attention interleaved per 512-token slice: attention
            # for q-slice ts starts as soon as q/k/v up to slice ts exist,
            # overlapping its ACT/DVE work with the next slice's qkv matmuls.
            def qkv_slice(ts):
                for pair in range(2):
                    for dst_t, wsb in ((qT[pair], wq_s), (kT[pair], wk_s)):
                        ps = psmm.tile([P, 512], f32, tag="mm")
                        for s in range(CS):
                            nc.tensor.matmul(
                                ps[:],
                                lhsT=wsb[:, s, pair * P:(pair + 1) * P],
                                rhs=xlt[:, s, ts * 512:(ts + 1) * 512],
                                start=(s == 0), stop=(s == CS - 1))
                        nc.scalar.copy(dst_t[:, ts * 512:(ts + 1) * 512],
                                       ps[:])
                for ti in range(4 * ts, 4 * ts + 4):
                    ps = psmm.tile([P, 512], f32, tag="mm")
                    for s in range(CS):
                        nc.tensor.matmul(
                            ps[:, :2 * P],
                            lhsT=xlt[:, s, ti * P:(ti + 1) * P],
                            rhs=wv_s[:, s, :],
                            start=(s == 0), stop=(s == CS - 1))
                    nc.vector.tensor_copy(
                        out=vA[:, ti, :, 0:64],
                        in_=ps[:, :2 * P].rearrange("p (h d) -> p h d",
                                                    h=HPC))

            for ts in range(NQ if do_p1 else 0):
                qkv_slice(ts)
                qs = ts
                for h in range(HPC):
                    pair, hp = h // 2, h % 2
                    yps = psy.tile([65, 512], f32, tag="yps")
                    nkt = 4 * qs + 4
                    for kt in range(nkt):
                        sps = psatt.tile([P, 512], f32, tag="satt")
                        nc.tensor.matmul(
                            sps[:],
                            lhsT=kT[pair][hp * 64:(hp + 1) * 64,
                                          kt * P:(kt + 1) * P],
                            rhs=qT[pair][hp * 64:(hp + 1) * 64,
                                         qs * 512:(qs + 1) * 512],
                            start=True, stop=True)
                        at = at_pool.tile([P, 512], bf16, tag="at")
                        nc.scalar.activation(at[:], sps[:], ACT.Exp)
                        j = kt - 4 * qs
                        if j >= 0:
                            nc.vector.tensor_mul(at[:], at[:],
                                                 masks[:, j, :])
                        nc.tensor.matmul(
                            yps[:], lhsT=vA[:, kt, h, :], rhs=at[:],
                            start=(kt == 0), stop=(kt == nkt - 1))
                    # normalize: ysb_h = y * (1/l), l = row 64 of yps
                    l64 = small.tile([65, 512], f32, tag="l64")
                    nc.vector.reciprocal(l64[64:65, :], yps[64:65, :])
                    r64 = small.tile([65, 512], bf16, tag="r64")
                    nc.vector.tensor_copy(out=r64[64:65, :],
                                          in_=l64[64:65, :])
                    rec = small.tile([1, 512], bf16, tag="rec")
                    nc.gpsimd.dma_start(rec[:], r64[64:65, :])
                    rps = psmm.tile([P, 512], f32, tag="mm")
                    nc.tensor.matmul(rps[:64, :], lhsT=ones64[:],
                                     rhs=rec[:], start=True, stop=True)
                    rsb = work.tile([64, 512], bf16, tag="rsb")
                    nc.scalar.copy(rsb[:], rps[:64, :])
                    nc.vector.tensor_mul(
                        ysb[h][:, qs * 512:(qs + 1) * 512],
                        yps[0:64, :], rsb[:])

            # ship Y to token owners: ybounce[j-block, h, :, :] = my head h
            # for rank j's tokens
            ybr = ybounce[:].rearrange("(j hh p) t -> j hh p t", j=NCORES,
                                       hh=HPC)
            for h in range(HPC if do_p1 else 0):
                for j in range(NCORES):
                    c = j % TPG
                    eng = nc.sync if j % 2 == 0 else nc.scalar
                    eng.dma_start(ybr[j, h, :, :],
                                  ysb[h][:, c * 512:(c + 1) * 512])

            if sim_mode:
                nc.sync.dma_start(a2a_out[0:256, :], ybounce[0:256, :])
                zf = work.tile([P, 512], bf16, tag="zfill")
                nc.vector.memset(zf[:], 0.0)
                for blk in range(2, 16):
                    nc.sync.dma_start(a2a_out[blk * P:(blk + 1) * P, :],
                                      zf[:])
            else:
                nc.gpsimd.collective_compute(
                    "AllToAll", mybir.AluOpType.bypass,
                    replica_groups=[list(range(NCORES))],
                    ins=[ybounce[:].opt()], outs=[a2a_out[:].opt()])

            if debug:
                nc.sync.dma_start(dbg["xlt"][:], xlt[:])
                nc.sync.dma_start(dbg["q0"][:], qT[0][:])
                nc.sync.dma_start(dbg["k0"][:], kT[0][:])
                nc.sync.dma_start(dbg["v"][:], vA[:])
                for h in range(HPC):
                    nc.sync.dma_start(dbg["y"][:, h, :], ysb[h][:])

        if not do_p2:
            with tc.tile_pool(name="dummy_out", bufs=1) as dpool:
                zt0 = dpool.tile([P, C], f32, tag="zt0")
                nc.vector.memset(zt0[:], 0.0)
                for i in range(NT2):
                    nc.sync.dma_start(out[i * P:(i + 1) * P, :], zt0[:])
            nc.compile()
            return nc

        # ================= phase 2: residual + LN2 + MLP ================
        with tc.tile_pool(name="mlp_per", bufs=1) as mper, \
             tc.tile_pool(name="psfc2", bufs=4, space="PSUM") as psfc2:

            x2 = mper.tile([P, NT2, C], f32, tag="x2")
            x2lt = mper.tile([P, CS, TCHUNK], bf16, tag="x2lt")
            hT = mper.tile([P, NS, TCHUNK], bf16, tag="hT")
            pw_s = mper.tile([P, 2 * CS, C], bf16, tag="pw")
            nc.gpsimd.dma_start(pw_s[:], pw[:])

            # all 8 recv blocks (2048 "channels") for my tokens
            ylt = mper.tile([P, 2 * CS, TCHUNK], bf16, tag="ylt")
            for s in range(2 * CS):
                engs[s % 4].dma_start(ylt[:, s, :],
                                      a2a_out[s * P:(s + 1) * P, :])

            # proj (zero-padded 2C contraction) + residual
            for i in range(0 if skip_proj else NT2):
                ct = work.tile([P, C], f32, tag="xbt")
                nc.sync.dma_start(ct[:], xc[i * P:(i + 1) * P, :])
                for n in range(2):
                    ps = psmm.tile([P, 512], f32, tag="mm")
                    for s in range(2 * CS):
                        nc.tensor.matmul(
                            ps[:],
                            lhsT=ylt[:, s, i * P:(i + 1) * P],
                            rhs=pw_s[:, s, n * 512:(n + 1) * 512],
                            start=(s == 0), stop=(s == 2 * CS - 1))
                    nc.vector.tensor_add(
                        x2[:, i, n * 512:(n + 1) * 512], ps[:],
                        ct[:, n * 512:(n + 1) * 512])

            if skip_proj:
                for i in range(NT2):
                    ct = work.tile([P, C], f32, tag="xbt")
                    nc.sync.dma_start(ct[:], xc[i * P:(i + 1) * P, :])
                    nc.vector.tensor_copy(out=x2[:, i, :], in_=ct[:])
            if skip_ln2:
                nc.vector.memset(x2lt[:], 0.001)
            else:
                layernorm_transpose(lambda i: x2[:, i, :], NT2, x2lt)

            # fc + gelu -> h^T (feature-major); fw streamed in 8 chunks
            if skip_fc:
                nc.vector.memset(hT[:], 0.001)
            for g in range(0 if skip_fc else 8):
                fwch = fw_chunks[g]
                if g >= 4:
                    engs[g % 4].dma_start(fwch[:], fw[g])
                for mq in range(4):
                    m = g * 4 + mq
                    ps = psmm.tile([P, 512], f32, tag="mm")
                    for s in range(CS):
                        nc.tensor.matmul(
                            ps[:],
                            lhsT=fwch[:, s, mq * P:(mq + 1) * P],
                            rhs=x2lt[:, s, :],
                            start=(s == 0), stop=(s == CS - 1))
                    nc.scalar.activation(hT[:, m, :], ps[:], ACT.Gelu)

            if debug:
                nc.sync.dma_start(dbg["x2"][:], x2[:])
                nc.sync.dma_start(dbg["ht"][:], hT[:])

            # fc2 + final residual (token-major out)
            if skip_fc2:
                for i in range(NT2):
                    ot2 = work.tile([P, C], f32, tag="xbt")
                    nc.vector.tensor_copy(out=ot2[:], in_=x2[:, i, :])
                    nc.sync.dma_start(out[i * P:(i + 1) * P, :], ot2[:])
            for n in range(0 if skip_fc2 else 2):
                pss = [psfc2.tile([P, 512], f32, tag="fc2", name=f"fc2_{n}_{t}")
                       for t in range(NT2)]
                for sg in range(NS // 4):
                    idx = n * 8 + sg
                    f2ch = f2_chunks[idx]
                    if idx >= 3:
                        engs[idx % 4].dma_start(f2ch[:], f2w[idx])
                    for sq in range(4):
                        s = 4 * sg + sq
                        for ti in range(NT2):
                            nc.tensor.matmul(
                                pss[ti][:],
                                lhsT=hT[:, s, ti * P:(ti + 1) * P],
                                rhs=f2ch[:, sq, :],
                                start=(s == 0), stop=(s == NS - 1))
                outt = [work.tile([P, C], f32, tag="ztw", name=f"ot_{n}_{t}")
                        for t in range(NT2)]
                for ti in range(NT2):
                    nc.vector.tensor_add(
                        outt[ti][:, n * 512:(n + 1) * 512], pss[ti][:],
                        x2[:, ti, n * 512:(n + 1) * 512])
                    nc.sync.dma_start(
                        out[ti * P:(ti + 1) * P, n * 512:(n + 1) * 512],
                        outt[ti][:, n * 512:(n + 1) * 512])

    nc.compile()
    return nc


def _prep_core_inputs(x, ln1_g, ln1_b, attn_w, attn_b, proj_w, proj_b,
                      ln2_g, ln2_b, fc_w, fc_b, fc2_w, fc2_b):
    """Host-side weight folding + per-core slicing. Returns in_maps list."""
    f = np.float32
    x = np.asarray(x, f)
    aw = np.asarray(ln1_g, f)[:, None] * np.asarray(attn_w, f)
    ab = np.asarray(attn_b, f) + np.asarray(ln1_b, f) @ np.asarray(attn_w, f)
    fwf = np.asarray(ln2_g, f)[:, None] * np.asarray(fc_w, f)
    fbf = np.asarray(fc_b, f) + np.asarray(ln2_b, f) @ np.asarray(fc_w, f)
    assert not np.any(ab) and not np.any(fbf), "nonzero qkv/fc bias unsupported"
    assert not np.any(np.asarray(proj_b, f)) and not np.any(
        np.asarray(fc2_b, f)), "nonzero proj/fc2 bias unsupported"

    qw = aw[:, :C] * f(1.0 / np.sqrt(HD))    # fold softmax scale into Wq
    kw = aw[:, C:2 * C]
    vw = aw[:, 2 * C:]
    pwf = np.asarray(proj_w, f)
    f2wf = np.asarray(fc2_w, f)

    def as_lhst(w):  # [K, N] -> [P, K//P, N]
        return np.ascontiguousarray(
            w.reshape(w.shape[0] // P, P, w.shape[1]).transpose(1, 0, 2)
        ).astype(BF)

    # per-group zero-padded proj weights for the 8-rank A2A recv layout:
    # recv-block j (rows 256j..256j+256) is core j's heads; valid iff core j
    # is in this core's batch group, and then equals proj_w rows for heads
    # 4*(j %% 4)..4*(j %% 4)+4.
    pw_pad = np.zeros((2, 2 * C, C), np.float32)
    for g in range(2):
        for j in range(NCORES):
            if j // TPG == g:
                r = j % TPG
                pw_pad[g, 256 * j:256 * (j + 1), :] = \
                    pwf[256 * r:256 * (r + 1), :]

    fw_l = as_lhst(fwf)            # [128, 8, 4096]
    fw_t = np.ascontiguousarray(
        np.stack([fw_l[:, :, g * 512:(g + 1) * 512] for g in range(8)]))
    f2_l = as_lhst(f2wf)           # [128, 32, 1024]
    f2w_t = np.ascontiguousarray(
        np.stack([f2_l[:, 4 * (i % 8):4 * (i % 8) + 4,
                       (i // 8) * 512:(i // 8 + 1) * 512]
                  for i in range(16)]))

    in_maps = []
    for core in range(NCORES):
        b, r = core // TPG, core % TPG
        cols = slice(256 * r, 256 * r + 256)
        in_maps.append({
            "xb": np.ascontiguousarray(x[b]).astype(BF),
            "xc": np.ascontiguousarray(x[b, TCHUNK * r:TCHUNK * (r + 1)]),
            "wq": as_lhst(qw[:, cols]),
            "wk": as_lhst(kw[:, cols]),
            "wv": as_lhst(vw[:, cols]),
            "pw": as_lhst(pw_pad[b]),
            "fw": fw_t,
            "f2w": f2w_t,
        })
    return in_maps


_built = {}


def run(inputs, trace=False, debug=DEBUG, **spmd_kwargs):
    key = ("dbg" if debug else "rel")
    if key not in _built:
        _built[key] = build_nc(debug=debug)
    nc = _built[key]
    in_maps = _prep_core_inputs(**inputs)
    res = run_bass_kernel_spmd(nc, in_maps, list(range(NCORES)),
                               trace=trace, **spmd_kwargs)
    full = np.empty((B, T, C), np.float32)
    for core in range(NCORES):
        b, r = core // TPG, core % TPG
        full[b, TCHUNK * r:TCHUNK * (r + 1)] = res.results[core]["out"]
    return full, res


def kernel(**inputs):
    full, _ = run(inputs, trace=False, debug=False)
    return full

